# revision 2
# baseline (speedup 1.0000x reference)
"""Trainium2 Bass kernel for nn_End2End_10316511445013 (embedding_lookup).

Math being implemented (see the reference nn.Module):
  1. x = logits + g,  g = -ln(-ln(u))          [B,L,V]
  2. In fp32 the straight-through one-hot  y = y_hard + y_soft - y_soft  is
     *exactly* alpha * one_hot(argmax(x)) with alpha = fl(fl(1+s)-s) = 1 +/- 2^-23,
     so the einsum with the embedding table is exactly an embedding row gather
     scaled by alpha (~1, error < 1.2e-7 relative -> we use 1).
  3. inputs_embeds[b,l] = att[b,l] * (idx < AV) * W[idx],  idx = argmax_v x[b,l,:]
  4. psg path: trunc_ids / flag index logic on [B,L] int tensors, then a second
     row gather of W, all computed on-device with small DVE ops + indirect DMA.

Distribution: data-parallel over the B*L = 2048 rows; 256 rows per core; the
94MB embedding table is replicated to every core.  Per core we stream the
(logits, gumbel) shard in [128, 4016] chunks: ACT computes a=Ln(u), b=Ln(-a);
one fused DVE tensor_tensor_reduce computes x = logits - b and the per-chunk
row max.  The argmax index is recovered by refetching only the winning chunk
per row (indirect DMA) and running max_index on it.
"""

import os
import sys
import tempfile

import numpy as np

sys.path.insert(0, "/opt/trn_rl_repo")

B, L, V, AV, D = 4, 512, 32128, 32000, 768
R = B * L            # 2048 tokens total
NCORES = 8
RC = R // NCORES     # 256 tokens per core
P = 128              # partitions
GROUPS = RC // P     # 2 groups of 128 tokens
NCH = 16             # vocab chunks per row (DMA/TT granularity)
C = V // NCH         # 2008
NSUB = 2             # max-reduce sub-chunks per chunk
RG = C // NSUB       # 1004: reduce granularity = phase-B refetch window
NCHR = NCH * NSUB    # 32 reduce chunks per row
NEG_BIG = -3.0e38
# The chunk-subtracts run on gpsimd (DVE 1-input reduces overlap them fully;
# DVE 2-input ops would serialize on the shared SBUF port) except the tail
# chunks, which go to DVE so gpsimd's queue drains before the kernel end.
DVE_TT_SLOTS = {5, 10, 15, 20, 25, 30}  # these chunk slots subtract on DVE

_CACHE = {}
LAST = {}            # exec_time_ns etc. for test harness introspection


def _build_program():
    from contextlib import ExitStack

    import concourse.bass as bass
    import concourse.tile as tile
    from concourse import bacc, mybir

    f32 = mybir.dt.float32
    i32 = mybir.dt.int32
    u32 = mybir.dt.uint32
    Alu = mybir.AluOpType
    Act = mybir.ActivationFunctionType

    nc = bacc.Bacc(
        "TRN2",
        target_bir_lowering=False,
        debug=False,
        enable_asserts=True,
        num_devices=NCORES,
    )

    lg_d = nc.dram_tensor("logits", [RC, V], f32, kind="ExternalInput")
    gu_d = nc.dram_tensor("gumbel", [RC, V], f32, kind="ExternalInput")
    w_d = nc.dram_tensor("wemb", [AV, D], f32, kind="ExternalInput")
    att_d = nc.dram_tensor("att", [B, L], i32, kind="ExternalInput")
    psg_d = nc.dram_tensor("psg", [B, L], i32, kind="ExternalInput")
    li_d = nc.dram_tensor("liota", [B, L], i32, kind="ExternalInput")
    bc_d = nc.dram_tensor("bcol", [RC, 1], i32, kind="ExternalInput")
    lc_d = nc.dram_tensor("lcol", [RC, 1], i32, kind="ExternalInput")
    lr_d = nc.dram_tensor("lrow", [RC, 1], i32, kind="ExternalInput")
    am_d = nc.dram_tensor("attmy", [RC, 1], i32, kind="ExternalInput")
    out_d = nc.dram_tensor("out", [RC, D], f32, kind="ExternalOutput")
    sc2_d = nc.dram_tensor("scratch2", [B, 2], i32, kind="Internal")

    # flat views for indirect row gathers (offset must be 0)
    lg_view = lg_d.ap().rearrange("r (n c) -> (r n) c", c=RG)
    gu_view = gu_d.ap().rearrange("r (n c) -> (r n) c", c=RG)
    att_flat = att_d.ap().rearrange("b (l o) -> (b l) o", o=1)
    psg_flat = psg_d.ap().rearrange("b (l o) -> (b l) o", o=1)

    with tile.TileContext(nc) as tc, ExitStack() as ctx:
        sm = ctx.enter_context(tc.tile_pool(name="small", bufs=1))
        lp = ctx.enter_context(tc.tile_pool(name="lg", bufs=6))
        up = ctx.enter_context(tc.tile_pool(name="gu", bufs=6))
        xp = ctx.enter_context(tc.tile_pool(name="x", bufs=4))
        rf = ctx.enter_context(tc.tile_pool(name="rf", bufs=2))
        ep = ctx.enter_context(tc.tile_pool(name="emb", bufs=1))
        tp = ctx.enter_context(tc.tile_pool(name="tok", bufs=2))

        # ---------------- psg index stage on [B, 512] ----------------
        A_t = sm.tile([B, L], i32, tag="psgA")
        nc.sync.dma_start(A_t[:], att_d.ap())
        P_t = sm.tile([B, L], i32, tag="psgP")
        nc.sync.dma_start(P_t[:], psg_d.ap())
        LI_t = sm.tile([B, L], i32, tag="psgLI")
        nc.sync.dma_start(LI_t[:], li_d.ap())

        shift = sm.tile([B, 1], i32, tag="shift")
        with nc.allow_low_precision(reason="exact int32 sum of 0/1 mask"):
            nc.vector.tensor_reduce(shift[:], A_t[:], mybir.AxisListType.X, Alu.add)

        FA = sm.tile([B, L], i32, tag="FA")  # FA[j] = att[511-j]
        nc.vector.tensor_copy(FA[:], A_t[:, ::-1])
        PR = sm.tile([B, L], i32, tag="PR")  # roll(psg,1) with [:,0]=1
        nc.vector.memset(PR[:, 0:1], 1)
        nc.vector.tensor_copy(PR[:, 1:L], P_t[:, 0 : L - 1])

        t1 = sm.tile([B, L], i32, tag="t1")
        nc.vector.tensor_scalar(t1[:], FA[:], 0, None, Alu.is_equal)
        t2 = sm.tile([B, L], i32, tag="t2")
        nc.vector.tensor_scalar(t2[:], PR[:], 0, None, Alu.not_equal)
        nzm = sm.tile([B, L], i32, tag="nzm")
        nc.vector.tensor_tensor(nzm[:], t1[:], t2[:], Alu.mult)

        # v(j) = (j + shift) & 511 : position in trunc space
        c511b = sm.tile([B, 1], i32, tag="c511b")
        nc.vector.memset(c511b[:], 511)
        v_t = sm.tile([B, L], i32, tag="v")
        nc.vector.tensor_tensor(
            v_t[:], LI_t[:], shift[:, 0:1].to_broadcast([B, L]), Alu.add
        )
        nc.vector.tensor_tensor(
            v_t[:], v_t[:], c511b[:, 0:1].to_broadcast([B, L]), Alu.bitwise_and
        )
        # cand = nz ? v : 9999  ==  (v - 9999)*nz + 9999
        c1 = sm.tile([B, L], i32, tag="c1")
        nc.vector.scalar_tensor_tensor(c1[:], v_t[:], 9999, nzm[:], Alu.subtract, Alu.mult)
        cand = sm.tile([B, L], i32, tag="cand")
        nc.vector.tensor_scalar(cand[:], c1[:], 9999, None, Alu.add)
        nzpos = sm.tile([B, 1], i32, tag="nzpos")
        nc.vector.tensor_reduce(nzpos[:], cand[:], mybir.AxisListType.X, Alu.min)

        s2t = sm.tile([B, 2], i32, tag="s2t")
        nc.vector.tensor_copy(s2t[:, 0:1], shift[:])
        nc.vector.tensor_copy(s2t[:, 1:2], nzpos[:])
        nc.sync.dma_start(sc2_d.ap(), s2t[:])

        ones_i = sm.tile([P, 1], i32, tag="ones")
        nc.vector.memset(ones_i[:], 1)
        c511p = sm.tile([P, 1], i32, tag="c511p")
        nc.vector.memset(c511p[:], 511)

        # ---------------- early psg token-side gathers (independent of phase A) --
        e2s, s2fs, s1parts = [], [], []
        for g in range(GROUPS):
            rows = slice(g * P, (g + 1) * P)
            bvec = tp.tile([P, 1], i32, tag="bvec")
            nc.sync.dma_start(bvec[:], bc_d.ap()[rows, :])
            lvec = tp.tile([P, 1], i32, tag="lvec")
            nc.sync.dma_start(lvec[:], lc_d.ap()[rows, :])
            sn = tp.tile([P, 2], i32, tag="sn")
            nc.gpsimd.indirect_dma_start(
                out=sn[:],
                out_offset=None,
                in_=sc2_d.ap(),
                in_offset=bass.IndirectOffsetOnAxis(ap=bvec[:, 0:1], axis=0),
            )
            # p = (l - shift + 512) & 511
            pv = tp.tile([P, 1], i32, tag="pv")
            nc.vector.tensor_tensor(pv[:], lvec[:], sn[:, 0:1], Alu.subtract)
            nc.vector.tensor_scalar(pv[:], pv[:], 512, None, Alu.add)
            nc.vector.tensor_tensor(pv[:], pv[:], c511p[:], Alu.bitwise_and)
            bsh = tp.tile([P, 1], i32, tag="bsh")
            nc.vector.tensor_scalar(bsh[:], bvec[:], 512, None, Alu.mult)
            # gather att[b, 511-p] : off = b*512 + 511 - p
            offa2 = tp.tile([P, 1], i32, tag="offa2")
            nc.vector.tensor_scalar(offa2[:], pv[:], -1, 511, Alu.mult, Alu.add)
            nc.vector.tensor_tensor(offa2[:], offa2[:], bsh[:], Alu.add)
            gA = tp.tile([P, 1], i32, tag="gA")
            nc.gpsimd.indirect_dma_start(
                out=gA[:],
                out_offset=None,
                in_=att_flat,
                in_offset=bass.IndirectOffsetOnAxis(ap=offa2[:, 0:1], axis=0),
            )
            # gather psg_input[b, p-1] (clamped; p==0 handled by select)
            offp = tp.tile([P, 1], i32, tag="offp")
            nc.vector.tensor_tensor(offp[:], bsh[:], pv[:], Alu.add)
            nc.vector.tensor_scalar(offp[:], offp[:], -1, 0, Alu.add, Alu.max)
            gP = tp.tile([P, 1], i32, tag="gP")
            nc.gpsimd.indirect_dma_start(
                out=gP[:],
                out_offset=None,
                in_=psg_flat,
                in_offset=bass.IndirectOffsetOnAxis(ap=offp[:, 0:1], axis=0),
            )
            eq0 = tp.tile([P, 1], i32, tag="eq0")
            nc.vector.tensor_scalar(eq0[:], pv[:], 0, None, Alu.is_equal)
            gPe = tp.tile([P, 1], i32, tag="gPe")
            nc.vector.select(gPe[:], eq0[:], ones_i[:], gP[:])
            tA = tp.tile([P, 1], i32, tag="tA")
            nc.vector.tensor_scalar(tA[:], gA[:], -1, 1, Alu.mult, Alu.add)
            id2 = tp.tile([P, 1], i32, tag="id2")
            nc.vector.tensor_tensor(id2[:], tA[:], gPe[:], Alu.mult)
            s2f = sm.tile([P, 1], f32, tag=f"s2f{g}")
            nc.vector.tensor_tensor(s2f[:], lvec[:], sn[:, 1:2], Alu.is_ge)
            e2 = sm.tile([P, D], f32, tag=f"e2_{g}")
            nc.gpsimd.indirect_dma_start(
                out=e2[:],
                out_offset=None,
                in_=w_d.ap(),
                in_offset=bass.IndirectOffsetOnAxis(ap=id2[:, 0:1], axis=0),
            )
            am_t = tp.tile([P, 1], i32, tag="am")
            nc.sync.dma_start(am_t[:], am_d.ap()[rows, :])
            attf = sm.tile([P, 1], f32, tag=f"attf{g}")
            nc.vector.tensor_copy(attf[:], am_t[:])
            e2s.append(e2)
            s2fs.append(s2f)
            s1parts.append(attf)

        # ---------------- phase A: stream chunks group-sequentially ----------------
        mchs = []
        for g in range(GROUPS):
            mch_g = sm.tile([P, NCHR], f32, tag=f"mch{g}")
            mchs.append(mch_g)
        for g in range(GROUPS):
            # ---- phase A chunks for this group ----
            for cc in range(NCH):
                rows = slice(g * P, (g + 1) * P)
                mch = mchs[g]
                lg_t = lp.tile([P, C], f32, tag="lg")
                nc.sync.dma_start(lg_t[:], lg_d.ap()[rows, cc * C : (cc + 1) * C])
                gu_t = up.tile([P, C], f32, tag="gu")
                nc.sync.dma_start(gu_t[:], gu_d.ap()[rows, cc * C : (cc + 1) * C])

                # in-place on ACT: u -> ln(u) -> ln(-ln(u)); subtract into a
                # separate x tile so the lg slot frees as soon as TT reads it
                nc.scalar.activation(gu_t[:], gu_t[:], Act.Ln)
                nc.scalar.activation(gu_t[:], gu_t[:], Act.Ln, scale=-1.0)
                slot = g * NCH + cc
                eng = nc.vector if slot in DVE_TT_SLOTS else nc.gpsimd
                x_t = xp.tile([P, C], f32, tag="x")
                eng.tensor_tensor(x_t[:], lg_t[:], gu_t[:], Alu.subtract)
                for ss in range(NSUB):
                    nc.vector.tensor_reduce(
                        mch[:, cc * NSUB + ss : cc * NSUB + ss + 1],
                        x_t[:, ss * RG : (ss + 1) * RG],
                        mybir.AxisListType.X,
                        Alu.max,
                    )

            # ---------------- phase B + gathers for this group ----------------
            rows = slice(g * P, (g + 1) * P)
            mch = mchs[g]

            # ---- winning chunk per row ----
            M_t = sm.tile([P, 1], f32, tag=f"M{g}")
            nc.vector.tensor_reduce(M_t[:], mch[:], mybir.AxisListType.X, Alu.max)
            M8 = sm.tile([P, 8], f32, tag=f"M8{g}")
            nc.vector.tensor_copy(M8[:], M_t[:, 0:1].to_broadcast([P, 8]))
            c8 = sm.tile([P, 8], u32, tag=f"c8{g}")
            nc.vector.max_index(c8[:], M8[:], mch[:])
            cst = sm.tile([P, 1], i32, tag=f"cst{g}")
            nc.vector.tensor_copy(cst[:], c8[:, 0:1])

            # ---- phase B: refetch winning chunk, exact argmax ----
            lr_t = tp.tile([P, 1], i32, tag="lr")
            nc.sync.dma_start(lr_t[:], lr_d.ap()[rows, :])
            offA = tp.tile([P, 1], i32, tag="offA")
            nc.vector.scalar_tensor_tensor(offA[:], lr_t[:], NCHR, cst[:], Alu.mult, Alu.add)

            lgr = rf.tile([P, RG], f32, tag="lgr")
            nc.gpsimd.indirect_dma_start(
                out=lgr[:],
                out_offset=None,
                in_=lg_view,
                in_offset=bass.IndirectOffsetOnAxis(ap=offA[:, 0:1], axis=0),
            )
            gur = rf.tile([P, RG], f32, tag="gur")
            nc.gpsimd.indirect_dma_start(
                out=gur[:],
                out_offset=None,
                in_=gu_view,
                in_offset=bass.IndirectOffsetOnAxis(ap=offA[:, 0:1], axis=0),
            )
            nc.scalar.activation(gur[:], gur[:], Act.Ln)
            nc.scalar.activation(gur[:], gur[:], Act.Ln, scale=-1.0)
            nc.vector.tensor_tensor(lgr[:], lgr[:], gur[:], Alu.subtract)
            li8 = sm.tile([P, 8], u32, tag=f"li8{g}")
            nc.vector.max_index(li8[:], M8[:], lgr[:])
            lii = sm.tile([P, 1], i32, tag=f"lii{g}")
            nc.vector.tensor_copy(lii[:], li8[:, 0:1])
            gidx = sm.tile([P, 1], i32, tag=f"gidx{g}")
            nc.vector.scalar_tensor_tensor(gidx[:], cst[:], RG, lii[:], Alu.mult, Alu.add)

            # ---- gather 1: argmax embedding ----
            v1f = tp.tile([P, 1], f32, tag="v1f")
            nc.vector.tensor_scalar(v1f[:], gidx[:], AV, None, Alu.is_lt)
            s1 = tp.tile([P, 1], f32, tag="s1")
            nc.vector.tensor_tensor(s1[:], v1f[:], s1parts[g][:], Alu.mult)
            idx1c = tp.tile([P, 1], i32, tag="idx1c")
            nc.vector.tensor_scalar(idx1c[:], gidx[:], AV - 1, None, Alu.min)
            e1 = ep.tile([P, D], f32, tag="e1")
            nc.gpsimd.indirect_dma_start(
                out=e1[:],
                out_offset=None,
                in_=w_d.ap(),
                in_offset=bass.IndirectOffsetOnAxis(ap=idx1c[:, 0:1], axis=0),
            )

            # ---- combine + store ----
            o1 = ep.tile([P, D], f32, tag="o1")
            nc.vector.tensor_scalar(o1[:], e1[:], s1[:, 0:1], None, Alu.mult)
            o2 = ep.tile([P, D], f32, tag="o2")
            nc.vector.scalar_tensor_tensor(
                o2[:], e2s[g][:], s2fs[g][:, 0:1], o1[:], Alu.mult, Alu.add
            )
            nc.sync.dma_start(out_d.ap()[rows, :], o2[:])

    nc.compile()
    return nc


def _get_program():
    if "nc" not in _CACHE:
        _CACHE["nc"] = _build_program()
    return _CACHE["nc"]


def make_in_maps(logits, gumbel_u, word_embeddings, rwrt_attention, psg_input):
    lg = np.ascontiguousarray(np.asarray(logits, np.float32).reshape(R, V))
    gu = np.ascontiguousarray(np.asarray(gumbel_u, np.float32).reshape(R, V))
    W = np.ascontiguousarray(np.asarray(word_embeddings, np.float32))
    att = np.ascontiguousarray(np.asarray(rwrt_attention, np.int32))
    psg = np.ascontiguousarray(np.asarray(psg_input, np.int32))
    liota = np.tile(np.arange(L, dtype=np.int32), (B, 1))
    att_flat = att.reshape(R)
    in_maps = []
    for c in range(NCORES):
        r0 = c * RC
        rows = np.arange(r0, r0 + RC, dtype=np.int32)
        in_maps.append(
            {
                "logits": lg[r0 : r0 + RC],
                "gumbel": gu[r0 : r0 + RC],
                "wemb": W,
                "att": att,
                "psg": psg,
                "liota": liota,
                "bcol": np.ascontiguousarray((rows >> 9).reshape(RC, 1)),
                "lcol": np.ascontiguousarray((rows & 511).reshape(RC, 1)),
                "lrow": np.arange(RC, dtype=np.int32).reshape(RC, 1),
                "attmy": np.ascontiguousarray(
                    att_flat[r0 : r0 + RC].reshape(RC, 1)
                ),
            }
        )
    return in_maps


def kernel(logits, gumbel_u, word_embeddings, rwrt_attention, psg_input):
    from concourse import bass_utils

    nc = _get_program()
    in_maps = make_in_maps(logits, gumbel_u, word_embeddings, rwrt_attention, psg_input)
    tmpdir = os.environ.get("BASS_KERNEL_TMPDIR") or None
    res = bass_utils.run_bass_kernel_spmd(
        nc, in_maps, core_ids=list(range(NCORES)), tmpdir=tmpdir
    )
    LAST["exec_time_ns"] = res.exec_time_ns
    LAST["tmpdir"] = tmpdir
    if res.instructions_and_trace is not None:
        LAST["trace_path"] = res.instructions_and_trace[1]
    out = np.concatenate([res.results[c]["out"] for c in range(NCORES)], axis=0)
    return out.reshape(B, L, D).astype(np.float32)



# revision 6
# speedup vs baseline: 1.0012x; 1.0012x over previous
"""Trainium2 Bass kernel for nn_End2End_10316511445013 (embedding_lookup).

Math being implemented (see the reference nn.Module):
  1. x = logits + g,  g = -ln(-ln(u))          [B,L,V]
  2. In fp32 the straight-through one-hot  y = y_hard + y_soft - y_soft  is
     *exactly* alpha * one_hot(argmax(x)) with alpha = fl(fl(1+s)-s) = 1 +/- 2^-23,
     so the einsum with the embedding table is exactly an embedding row gather
     scaled by alpha (~1, error < 1.2e-7 relative -> we use 1).
  3. inputs_embeds[b,l] = att[b,l] * (idx < AV) * W[idx],  idx = argmax_v x[b,l,:]
  4. psg path: trunc_ids / flag index logic on [B,L] int tensors, then a second
     row gather of W, all computed on-device with small DVE ops + indirect DMA.

Distribution: data-parallel over the B*L = 2048 rows; 256 rows per core; the
94MB embedding table is replicated to every core.  Per core we stream the
(logits, gumbel) shard in [128, 2008] chunks: ACT computes a=Ln(u), b=Ln(-a)
in place; a fused DVE tensor_tensor_reduce computes x = logits - b and the
per-502-window row max in a single DVE pass (keeps the Pool engine free for
SWDGE gathers and leaves DMA as the only saturated resource).  The argmax
index is recovered by refetching only the winning 502-wide window per row
(indirect DMA) and running max_index on it.
"""

import os
import sys

import numpy as np

sys.path.insert(0, "/opt/trn_rl_repo")

B, L, V, AV, D = 4, 512, 32128, 32000, 768
R = B * L            # 2048 tokens total
NCORES = 8
RC = R // NCORES     # 256 tokens per core
P = 128              # partitions
GROUPS = RC // P     # 2 groups of 128 tokens
NCH = 16             # vocab chunks per row (DMA granularity)
C = V // NCH         # 2008
NSUB = 4             # max-reduce windows per chunk
RG = C // NSUB       # 502: reduce granularity = phase-B refetch window
NCHR = NCH * NSUB    # 64 reduce windows per row
NEG_BIG = -3.0e38
USE_TTR = os.environ.get("KERNEL_NO_TTR", "") == ""

_CACHE = {}
LAST = {}            # exec_time_ns etc. for test harness introspection


def _build_program():
    from contextlib import ExitStack

    import concourse.bass as bass
    import concourse.tile as tile
    from concourse import bacc, mybir

    f32 = mybir.dt.float32
    i32 = mybir.dt.int32
    u32 = mybir.dt.uint32
    Alu = mybir.AluOpType
    Act = mybir.ActivationFunctionType

    nc = bacc.Bacc(
        "TRN2",
        target_bir_lowering=False,
        debug=False,
        enable_asserts=True,
        num_devices=NCORES,
    )

    lg_d = nc.dram_tensor("logits", [RC, V], f32, kind="ExternalInput")
    gu_d = nc.dram_tensor("gumbel", [RC, V], f32, kind="ExternalInput")
    w_d = nc.dram_tensor("wemb", [AV, D], f32, kind="ExternalInput")
    att_d = nc.dram_tensor("att", [B, L], i32, kind="ExternalInput")
    psg_d = nc.dram_tensor("psg", [B, L], i32, kind="ExternalInput")
    li_d = nc.dram_tensor("liota", [B, L], i32, kind="ExternalInput")
    bc_d = nc.dram_tensor("bcol", [RC, 1], i32, kind="ExternalInput")
    lc_d = nc.dram_tensor("lcol", [RC, 1], i32, kind="ExternalInput")
    lr_d = nc.dram_tensor("lrow", [RC, 1], i32, kind="ExternalInput")
    am_d = nc.dram_tensor("attmy", [RC, 1], i32, kind="ExternalInput")
    out_d = nc.dram_tensor("out", [RC, D], f32, kind="ExternalOutput")
    sc2_d = nc.dram_tensor("scratch2", [B, 2], i32, kind="Internal")

    # flat views for indirect row gathers (offset must be 0)
    lg_view = lg_d.ap().rearrange("r (n c) -> (r n) c", c=RG)
    gu_view = gu_d.ap().rearrange("r (n c) -> (r n) c", c=RG)
    att_flat = att_d.ap().rearrange("b (l o) -> (b l) o", o=1)
    psg_flat = psg_d.ap().rearrange("b (l o) -> (b l) o", o=1)

    with tile.TileContext(nc) as tc, ExitStack() as ctx:
        sm = ctx.enter_context(tc.tile_pool(name="small", bufs=1))
        lp = ctx.enter_context(tc.tile_pool(name="lg", bufs=6))
        up = ctx.enter_context(tc.tile_pool(name="gu", bufs=6))
        xp = ctx.enter_context(tc.tile_pool(name="x", bufs=3))
        rf = ctx.enter_context(tc.tile_pool(name="rf", bufs=2))
        ep = ctx.enter_context(tc.tile_pool(name="emb", bufs=1))
        tp = ctx.enter_context(tc.tile_pool(name="tok", bufs=2))

        # ---------------- psg index stage on [B, 512] ----------------
        # All small loads go through Pool SWDGE so the SP HWDGE queue stays
        # exclusively on the big streaming chunk DMAs.
        A_t = sm.tile([B, L], i32, tag="psgA")
        nc.sync.dma_start(A_t[:], att_d.ap())
        P_t = sm.tile([B, L], i32, tag="psgP")
        nc.sync.dma_start(P_t[:], psg_d.ap())
        LI_t = sm.tile([B, L], i32, tag="psgLI")
        nc.sync.dma_start(LI_t[:], li_d.ap())

        shift = sm.tile([B, 1], i32, tag="shift")
        with nc.allow_low_precision(reason="exact int32 sum of 0/1 mask"):
            nc.vector.tensor_reduce(shift[:], A_t[:], mybir.AxisListType.X, Alu.add)

        FA = sm.tile([B, L], i32, tag="FA")  # FA[j] = att[511-j]
        nc.vector.tensor_copy(FA[:], A_t[:, ::-1])
        PR = sm.tile([B, L], i32, tag="PR")  # roll(psg,1) with [:,0]=1
        nc.vector.memset(PR[:, 0:1], 1)
        nc.vector.tensor_copy(PR[:, 1:L], P_t[:, 0 : L - 1])

        t1 = sm.tile([B, L], i32, tag="t1")
        nc.vector.tensor_scalar(t1[:], FA[:], 0, None, Alu.is_equal)
        t2 = sm.tile([B, L], i32, tag="t2")
        nc.vector.tensor_scalar(t2[:], PR[:], 0, None, Alu.not_equal)
        nzm = sm.tile([B, L], i32, tag="nzm")
        nc.vector.tensor_tensor(nzm[:], t1[:], t2[:], Alu.mult)

        # v(j) = (j + shift) & 511 : position in trunc space
        c511b = sm.tile([B, 1], i32, tag="c511b")
        nc.vector.memset(c511b[:], 511)
        v_t = sm.tile([B, L], i32, tag="v")
        nc.vector.tensor_tensor(
            v_t[:], LI_t[:], shift[:, 0:1].to_broadcast([B, L]), Alu.add
        )
        nc.vector.tensor_tensor(
            v_t[:], v_t[:], c511b[:, 0:1].to_broadcast([B, L]), Alu.bitwise_and
        )
        # cand = nz ? v : 9999  ==  (v - 9999)*nz + 9999
        c1 = sm.tile([B, L], i32, tag="c1")
        nc.vector.scalar_tensor_tensor(c1[:], v_t[:], 9999, nzm[:], Alu.subtract, Alu.mult)
        cand = sm.tile([B, L], i32, tag="cand")
        nc.vector.tensor_scalar(cand[:], c1[:], 9999, None, Alu.add)
        nzpos = sm.tile([B, 1], i32, tag="nzpos")
        nc.vector.tensor_reduce(nzpos[:], cand[:], mybir.AxisListType.X, Alu.min)

        s2t = sm.tile([B, 2], i32, tag="s2t")
        nc.vector.tensor_copy(s2t[:, 0:1], shift[:])
        nc.vector.tensor_copy(s2t[:, 1:2], nzpos[:])
        nc.sync.dma_start(sc2_d.ap(), s2t[:])

        ones_i = sm.tile([P, 1], i32, tag="ones")
        nc.vector.memset(ones_i[:], 1)
        c511p = sm.tile([P, 1], i32, tag="c511p")
        nc.vector.memset(c511p[:], 511)

        # ---------------- early psg token-side gathers (independent of phase A) --
        e2s, s2fs, s1parts = [], [], []
        for g in range(GROUPS):
            rows = slice(g * P, (g + 1) * P)
            bvec = tp.tile([P, 1], i32, tag="bvec")
            nc.sync.dma_start(bvec[:], bc_d.ap()[rows, :])
            lvec = tp.tile([P, 1], i32, tag="lvec")
            nc.sync.dma_start(lvec[:], lc_d.ap()[rows, :])
            sn = tp.tile([P, 2], i32, tag="sn")
            nc.gpsimd.indirect_dma_start(
                out=sn[:],
                out_offset=None,
                in_=sc2_d.ap(),
                in_offset=bass.IndirectOffsetOnAxis(ap=bvec[:, 0:1], axis=0),
            )
            # p = (l - shift + 512) & 511
            pv = tp.tile([P, 1], i32, tag="pv")
            nc.vector.tensor_tensor(pv[:], lvec[:], sn[:, 0:1], Alu.subtract)
            nc.vector.tensor_scalar(pv[:], pv[:], 512, None, Alu.add)
            nc.vector.tensor_tensor(pv[:], pv[:], c511p[:], Alu.bitwise_and)
            bsh = tp.tile([P, 1], i32, tag="bsh")
            nc.vector.tensor_scalar(bsh[:], bvec[:], 512, None, Alu.mult)
            # gather att[b, 511-p] : off = b*512 + 511 - p
            offa2 = tp.tile([P, 1], i32, tag="offa2")
            nc.vector.tensor_scalar(offa2[:], pv[:], -1, 511, Alu.mult, Alu.add)
            nc.vector.tensor_tensor(offa2[:], offa2[:], bsh[:], Alu.add)
            gA = tp.tile([P, 1], i32, tag="gA")
            nc.gpsimd.indirect_dma_start(
                out=gA[:],
                out_offset=None,
                in_=att_flat,
                in_offset=bass.IndirectOffsetOnAxis(ap=offa2[:, 0:1], axis=0),
            )
            # gather psg_input[b, p-1] (clamped; p==0 handled by select)
            offp = tp.tile([P, 1], i32, tag="offp")
            nc.vector.tensor_tensor(offp[:], bsh[:], pv[:], Alu.add)
            nc.vector.tensor_scalar(offp[:], offp[:], -1, 0, Alu.add, Alu.max)
            gP = tp.tile([P, 1], i32, tag="gP")
            nc.gpsimd.indirect_dma_start(
                out=gP[:],
                out_offset=None,
                in_=psg_flat,
                in_offset=bass.IndirectOffsetOnAxis(ap=offp[:, 0:1], axis=0),
            )
            eq0 = tp.tile([P, 1], i32, tag="eq0")
            nc.vector.tensor_scalar(eq0[:], pv[:], 0, None, Alu.is_equal)
            gPe = tp.tile([P, 1], i32, tag="gPe")
            nc.vector.select(gPe[:], eq0[:], ones_i[:], gP[:])
            tA = tp.tile([P, 1], i32, tag="tA")
            nc.vector.tensor_scalar(tA[:], gA[:], -1, 1, Alu.mult, Alu.add)
            id2 = tp.tile([P, 1], i32, tag="id2")
            nc.vector.tensor_tensor(id2[:], tA[:], gPe[:], Alu.mult)
            s2f = sm.tile([P, 1], f32, tag=f"s2f{g}")
            nc.vector.tensor_tensor(s2f[:], lvec[:], sn[:, 1:2], Alu.is_ge)
            e2 = sm.tile([P, D], f32, tag=f"e2_{g}")
            nc.gpsimd.indirect_dma_start(
                out=e2[:],
                out_offset=None,
                in_=w_d.ap(),
                in_offset=bass.IndirectOffsetOnAxis(ap=id2[:, 0:1], axis=0),
            )
            am_t = tp.tile([P, 1], i32, tag="am")
            nc.sync.dma_start(am_t[:], am_d.ap()[rows, :])
            attf = sm.tile([P, 1], f32, tag=f"attf{g}")
            nc.vector.tensor_copy(attf[:], am_t[:])
            e2s.append(e2)
            s2fs.append(s2f)
            s1parts.append(attf)

        # ---------------- phase A: stream chunks group-sequentially ----------------
        mchs = []
        for g in range(GROUPS):
            mch_g = sm.tile([P, NCHR], f32, tag=f"mch{g}")
            mchs.append(mch_g)
        for g in range(GROUPS):
            # ---- phase A chunks for this group ----
            for cc in range(NCH):
                rows = slice(g * P, (g + 1) * P)
                mch = mchs[g]
                lg_t = lp.tile([P, C], f32, tag="lg")
                nc.sync.dma_start(lg_t[:], lg_d.ap()[rows, cc * C : (cc + 1) * C])
                gu_t = up.tile([P, C], f32, tag="gu")
                nc.sync.dma_start(gu_t[:], gu_d.ap()[rows, cc * C : (cc + 1) * C])

                # in-place on ACT: u -> ln(u) -> ln(-ln(u))
                nc.scalar.activation(gu_t[:], gu_t[:], Act.Ln)
                nc.scalar.activation(gu_t[:], gu_t[:], Act.Ln, scale=-1.0)
                x_t = xp.tile([P, C], f32, tag="x")
                if USE_TTR:
                    # fused x = lg - gu and windowed row-max, one DVE pass/window
                    for ss in range(NSUB):
                        w0 = ss * RG
                        nc.vector.tensor_tensor_reduce(
                            out=x_t[:, w0 : w0 + RG],
                            in0=lg_t[:, w0 : w0 + RG],
                            in1=gu_t[:, w0 : w0 + RG],
                            scale=1.0,
                            scalar=NEG_BIG,
                            op0=Alu.subtract,
                            op1=Alu.max,
                            accum_out=mch[:, cc * NSUB + ss : cc * NSUB + ss + 1],
                        )
                else:
                    nc.vector.tensor_tensor(x_t[:], lg_t[:], gu_t[:], Alu.subtract)
                    for ss in range(NSUB):
                        w0 = ss * RG
                        nc.vector.tensor_reduce(
                            mch[:, cc * NSUB + ss : cc * NSUB + ss + 1],
                            x_t[:, w0 : w0 + RG],
                            mybir.AxisListType.X,
                            Alu.max,
                        )

            # ---------------- phase B + gathers for this group ----------------
            rows = slice(g * P, (g + 1) * P)
            mch = mchs[g]

            # ---- winning window per row ----
            M_t = sm.tile([P, 1], f32, tag=f"M{g}")
            nc.vector.tensor_reduce(M_t[:], mch[:], mybir.AxisListType.X, Alu.max)
            M8 = sm.tile([P, 8], f32, tag=f"M8{g}")
            nc.vector.tensor_copy(M8[:], M_t[:, 0:1].to_broadcast([P, 8]))
            c8 = sm.tile([P, 8], u32, tag=f"c8{g}")
            nc.vector.max_index(c8[:], M8[:], mch[:])
            cst = sm.tile([P, 1], i32, tag=f"cst{g}")
            nc.vector.tensor_copy(cst[:], c8[:, 0:1])

            # ---- phase B: refetch winning window, exact argmax ----
            lr_t = tp.tile([P, 1], i32, tag="lr")
            nc.sync.dma_start(lr_t[:], lr_d.ap()[rows, :])
            offA = tp.tile([P, 1], i32, tag="offA")
            nc.vector.scalar_tensor_tensor(offA[:], lr_t[:], NCHR, cst[:], Alu.mult, Alu.add)

            lgr = rf.tile([P, RG], f32, tag="lgr")
            nc.gpsimd.indirect_dma_start(
                out=lgr[:],
                out_offset=None,
                in_=lg_view,
                in_offset=bass.IndirectOffsetOnAxis(ap=offA[:, 0:1], axis=0),
            )
            gur = rf.tile([P, RG], f32, tag="gur")
            nc.gpsimd.indirect_dma_start(
                out=gur[:],
                out_offset=None,
                in_=gu_view,
                in_offset=bass.IndirectOffsetOnAxis(ap=offA[:, 0:1], axis=0),
            )
            nc.scalar.activation(gur[:], gur[:], Act.Ln)
            nc.scalar.activation(gur[:], gur[:], Act.Ln, scale=-1.0)
            nc.vector.tensor_tensor(lgr[:], lgr[:], gur[:], Alu.subtract)
            li8 = sm.tile([P, 8], u32, tag=f"li8{g}")
            nc.vector.max_index(li8[:], M8[:], lgr[:])
            lii = sm.tile([P, 1], i32, tag=f"lii{g}")
            nc.vector.tensor_copy(lii[:], li8[:, 0:1])
            gidx = sm.tile([P, 1], i32, tag=f"gidx{g}")
            nc.vector.scalar_tensor_tensor(gidx[:], cst[:], RG, lii[:], Alu.mult, Alu.add)

            # ---- gather 1: argmax embedding ----
            v1f = tp.tile([P, 1], f32, tag="v1f")
            nc.vector.tensor_scalar(v1f[:], gidx[:], AV, None, Alu.is_lt)
            s1 = tp.tile([P, 1], f32, tag="s1")
            nc.vector.tensor_tensor(s1[:], v1f[:], s1parts[g][:], Alu.mult)
            idx1c = tp.tile([P, 1], i32, tag="idx1c")
            nc.vector.tensor_scalar(idx1c[:], gidx[:], AV - 1, None, Alu.min)
            e1 = ep.tile([P, D], f32, tag="e1")
            nc.gpsimd.indirect_dma_start(
                out=e1[:],
                out_offset=None,
                in_=w_d.ap(),
                in_offset=bass.IndirectOffsetOnAxis(ap=idx1c[:, 0:1], axis=0),
            )

            # ---- combine + store ----
            o1 = ep.tile([P, D], f32, tag="o1")
            nc.vector.tensor_scalar(o1[:], e1[:], s1[:, 0:1], None, Alu.mult)
            o2 = ep.tile([P, D], f32, tag="o2")
            nc.vector.scalar_tensor_tensor(
                o2[:], e2s[g][:], s2fs[g][:, 0:1], o1[:], Alu.mult, Alu.add
            )
            nc.sync.dma_start(out_d.ap()[rows, :], o2[:])

    nc.compile()
    return nc


def _get_program():
    if "nc" not in _CACHE:
        _CACHE["nc"] = _build_program()
    return _CACHE["nc"]


def make_in_maps(logits, gumbel_u, word_embeddings, rwrt_attention, psg_input):
    lg = np.ascontiguousarray(np.asarray(logits, np.float32).reshape(R, V))
    gu = np.ascontiguousarray(np.asarray(gumbel_u, np.float32).reshape(R, V))
    W = np.ascontiguousarray(np.asarray(word_embeddings, np.float32))
    att = np.ascontiguousarray(np.asarray(rwrt_attention, np.int32))
    psg = np.ascontiguousarray(np.asarray(psg_input, np.int32))
    liota = np.tile(np.arange(L, dtype=np.int32), (B, 1))
    att_flat = att.reshape(R)
    in_maps = []
    for c in range(NCORES):
        r0 = c * RC
        rows = np.arange(r0, r0 + RC, dtype=np.int32)
        in_maps.append(
            {
                "logits": lg[r0 : r0 + RC],
                "gumbel": gu[r0 : r0 + RC],
                "wemb": W,
                "att": att,
                "psg": psg,
                "liota": liota,
                "bcol": np.ascontiguousarray((rows >> 9).reshape(RC, 1)),
                "lcol": np.ascontiguousarray((rows & 511).reshape(RC, 1)),
                "lrow": np.arange(RC, dtype=np.int32).reshape(RC, 1),
                "attmy": np.ascontiguousarray(
                    att_flat[r0 : r0 + RC].reshape(RC, 1)
                ),
            }
        )
    return in_maps


def kernel(logits, gumbel_u, word_embeddings, rwrt_attention, psg_input):
    from concourse import bass_utils

    nc = _get_program()
    in_maps = make_in_maps(logits, gumbel_u, word_embeddings, rwrt_attention, psg_input)
    tmpdir = os.environ.get("BASS_KERNEL_TMPDIR") or None
    res = bass_utils.run_bass_kernel_spmd(
        nc, in_maps, core_ids=list(range(NCORES)), tmpdir=tmpdir
    )
    LAST["exec_time_ns"] = res.exec_time_ns
    LAST["tmpdir"] = tmpdir
    if res.instructions_and_trace is not None:
        LAST["trace_path"] = res.instructions_and_trace[1]
    out = np.concatenate([res.results[c]["out"] for c in range(NCORES)], axis=0)
    return out.reshape(B, L, D).astype(np.float32)


# revision 13
# speedup vs baseline: 1.1345x; 1.1332x over previous
"""Trainium2 Bass kernel for nn_End2End_10316511445013 (embedding_lookup).

Math being implemented (see the reference nn.Module):
  1. x = logits + g,  g = -ln(-ln(u))          [B,L,V]
  2. In fp32 the straight-through one-hot  y = y_hard + y_soft - y_soft  is
     *exactly* alpha * one_hot(argmax(x)) with alpha = fl(fl(1+s)-s) = 1 +/- 2^-23,
     so the einsum with the embedding table is exactly an embedding row gather
     scaled by alpha (~1, error < 1.2e-7 relative -> we use 1).
  3. inputs_embeds[b,l] = att[b,l] * (idx < AV) * W[idx],  idx = argmax_v x[b,l,:]
  4. psg path: trunc_ids / flag index logic on [B,L] int tensors, then a second
     row gather of W, all computed on-device with small DVE ops + indirect DMA.

Distribution: data-parallel over the B*L = 2048 rows; 256 rows per core; the
94MB embedding table is replicated to every core.  Per core we stream the
(logits, gumbel) shard in [128, 2008] chunks.  Engine assignment keeps DMA the
only saturated resource:
  SP (sync)  : only the 64 streaming chunk DMAs + the 2 output stores (emitted
               last), so its in-order HWDGE queue never head-blocks the stream.
  ACT        : u -> ln(u) -> ln(-ln(u)) in place (2 passes per chunk).
  DVE        : x = logits - ln(-ln(u)) subtract, psg index math, max_index.
  Pool       : pool_max over [128, 4, 502] windows -> per-window row max, plus
               all SWDGE small loads / indirect gathers / output stores.
The last chunk of each group is split into 4 window-sized pieces so the
pipeline drain latency at the group boundary / kernel tail is ~3us not ~10us.
The argmax index is recovered by refetching only the winning 502-wide window
per row (indirect DMA) and running max_index on it.
"""

import os
import sys

import numpy as np

sys.path.insert(0, "/opt/trn_rl_repo")

B, L, V, AV, D = 4, 512, 32128, 32000, 768
R = B * L            # 2048 tokens total
NCORES = 8
RC = R // NCORES     # 256 tokens per core
P = 128              # partitions
GROUPS = RC // P     # 2 groups of 128 tokens
NCH = 16             # vocab chunks per row (DMA granularity)
C = V // NCH         # 2008
NSUB = 4             # max-reduce windows per chunk
RG = C // NSUB       # 502: reduce granularity = phase-B refetch window
NCHR = NCH * NSUB    # 64 reduce windows per row
NEG_BIG = -3.0e38
# windowed reduce: "dve3" = one 3D tensor_reduce per chunk, "dve" = per-window
RED = os.environ.get("KERNEL_RED", "dve3")
# full chunks whose subtract runs on DVE instead of Pool (Pool absorbs the
# rest; these slots sit where Pool is busy with SWDGE bursts)
DVE_SUB = {0, 3, 6, 9, 12}

_CACHE = {}
LAST = {}            # exec_time_ns etc. for test harness introspection


def _build_program():
    from contextlib import ExitStack

    import concourse.bass as bass
    import concourse.tile as tile
    from concourse import bacc, mybir

    f32 = mybir.dt.float32
    i32 = mybir.dt.int32
    u32 = mybir.dt.uint32
    Alu = mybir.AluOpType
    Act = mybir.ActivationFunctionType

    nc = bacc.Bacc(
        "TRN2",
        target_bir_lowering=False,
        debug=False,
        enable_asserts=True,
        num_devices=NCORES,
    )

    lg_d = nc.dram_tensor("logits", [RC, V], f32, kind="ExternalInput")
    gu_d = nc.dram_tensor("gumbel", [RC, V], f32, kind="ExternalInput")
    w_d = nc.dram_tensor("wemb", [AV, D], f32, kind="ExternalInput")
    att_d = nc.dram_tensor("att", [B, L], i32, kind="ExternalInput")
    psg_d = nc.dram_tensor("psg", [B, L], i32, kind="ExternalInput")
    bc_d = nc.dram_tensor("bcol", [RC, 1], i32, kind="ExternalInput")
    lc_d = nc.dram_tensor("lcol", [RC, 1], i32, kind="ExternalInput")
    lr_d = nc.dram_tensor("lrow", [RC, 1], i32, kind="ExternalInput")
    am_d = nc.dram_tensor("attmy", [RC, 1], i32, kind="ExternalInput")
    out_d = nc.dram_tensor("out", [RC, D], f32, kind="ExternalOutput")
    sc2_d = nc.dram_tensor("scratch2", [B, 2], i32, kind="Internal")

    # flat views for indirect row gathers (offset must be 0)
    lg_view = lg_d.ap().rearrange("r (n c) -> (r n) c", c=RG)
    gu_view = gu_d.ap().rearrange("r (n c) -> (r n) c", c=RG)
    att_flat = att_d.ap().rearrange("b (l o) -> (b l) o", o=1)
    psg_flat = psg_d.ap().rearrange("b (l o) -> (b l) o", o=1)

    with tile.TileContext(nc) as tc, ExitStack() as ctx:
        sm = ctx.enter_context(tc.tile_pool(name="small", bufs=1))
        lp = ctx.enter_context(tc.tile_pool(name="lg", bufs=6))
        up = ctx.enter_context(tc.tile_pool(name="gu", bufs=6))
        xp = ctx.enter_context(tc.tile_pool(name="x", bufs=4))
        sp2 = ctx.enter_context(tc.tile_pool(name="spl", bufs=3))
        rf = ctx.enter_context(tc.tile_pool(name="rf", bufs=2))
        ep = ctx.enter_context(tc.tile_pool(name="emb", bufs=2))
        tp = ctx.enter_context(tc.tile_pool(name="tok", bufs=2))

        # ---------------- small input loads: Pool SWDGE, no waits ----------
        A_t = sm.tile([B, L], i32, tag="psgA")
        nc.gpsimd.dma_start(A_t[:], att_d.ap())
        P_t = sm.tile([B, L], i32, tag="psgP")
        nc.gpsimd.dma_start(P_t[:], psg_d.ap())
        bvecs, lvecs, ams, lrs = [], [], [], []
        for g in range(GROUPS):
            rows = slice(g * P, (g + 1) * P)
            bvec = sm.tile([P, 1], i32, tag=f"bvec{g}")
            nc.gpsimd.dma_start(bvec[:], bc_d.ap()[rows, :])
            lvec = sm.tile([P, 1], i32, tag=f"lvec{g}")
            nc.gpsimd.dma_start(lvec[:], lc_d.ap()[rows, :])
            am_t = sm.tile([P, 1], i32, tag=f"am{g}")
            nc.gpsimd.dma_start(am_t[:], am_d.ap()[rows, :])
            lr_t = sm.tile([P, 1], i32, tag=f"lr{g}")
            nc.gpsimd.dma_start(lr_t[:], lr_d.ap()[rows, :])
            bvecs.append(bvec)
            lvecs.append(lvec)
            ams.append(am_t)
            lrs.append(lr_t)

        # ---------------- psg index stage on [B, 512] (DVE) ----------------
        LI_t = sm.tile([B, L], i32, tag="psgLI")
        nc.gpsimd.iota(LI_t[:], [[1, L]], base=0, channel_multiplier=0)

        shift = sm.tile([B, 1], i32, tag="shift")
        with nc.allow_low_precision(reason="exact int32 sum of 0/1 mask"):
            nc.vector.tensor_reduce(shift[:], A_t[:], mybir.AxisListType.X, Alu.add)

        FA = sm.tile([B, L], i32, tag="FA")  # FA[j] = att[511-j]
        nc.vector.tensor_copy(FA[:], A_t[:, ::-1])
        PR = sm.tile([B, L], i32, tag="PR")  # roll(psg,1) with [:,0]=1
        nc.vector.memset(PR[:, 0:1], 1)
        nc.vector.tensor_copy(PR[:, 1:L], P_t[:, 0 : L - 1])

        t1 = sm.tile([B, L], i32, tag="t1")
        nc.vector.tensor_scalar(t1[:], FA[:], 0, None, Alu.is_equal)
        t2 = sm.tile([B, L], i32, tag="t2")
        nc.vector.tensor_scalar(t2[:], PR[:], 0, None, Alu.not_equal)
        nzm = sm.tile([B, L], i32, tag="nzm")
        nc.vector.tensor_tensor(nzm[:], t1[:], t2[:], Alu.mult)

        # v(j) = (j + shift) & 511 : position in trunc space
        c511b = sm.tile([B, 1], i32, tag="c511b")
        nc.vector.memset(c511b[:], 511)
        v_t = sm.tile([B, L], i32, tag="v")
        nc.vector.tensor_tensor(
            v_t[:], LI_t[:], shift[:, 0:1].to_broadcast([B, L]), Alu.add
        )
        nc.vector.tensor_tensor(
            v_t[:], v_t[:], c511b[:, 0:1].to_broadcast([B, L]), Alu.bitwise_and
        )
        # cand = nz ? v : 9999  ==  (v - 9999)*nz + 9999
        c1 = sm.tile([B, L], i32, tag="c1")
        nc.vector.scalar_tensor_tensor(c1[:], v_t[:], 9999, nzm[:], Alu.subtract, Alu.mult)
        cand = sm.tile([B, L], i32, tag="cand")
        nc.vector.tensor_scalar(cand[:], c1[:], 9999, None, Alu.add)
        nzpos = sm.tile([B, 1], i32, tag="nzpos")
        nc.vector.tensor_reduce(nzpos[:], cand[:], mybir.AxisListType.X, Alu.min)

        s2t = sm.tile([B, 2], i32, tag="s2t")
        nc.vector.tensor_copy(s2t[:, 0:1], shift[:])
        nc.vector.tensor_copy(s2t[:, 1:2], nzpos[:])

        ones_i = sm.tile([P, 1], i32, tag="ones")
        nc.vector.memset(ones_i[:], 1)
        c511p = sm.tile([P, 1], i32, tag="c511p")
        nc.vector.memset(c511p[:], 511)

        mchs = []
        for g in range(GROUPS):
            mch_g = sm.tile([P, NCHR], f32, tag=f"mch{g}")
            mchs.append(mch_g)

        e2s, s2fs, s1parts, o2s = [], [], [], []

        def emit_chunk(g, cc):
            """Stream + process one [128, 2008] chunk; the last chunk of a
            group is split into 4 window-sized pieces for drain latency.
            The subtract runs on Pool (software TT) for most chunks and on
            DVE for DVE_SUB chunks + the split pieces, so neither engine's
            per-slot work exceeds the 5.7us DMA slot."""
            rows = slice(g * P, (g + 1) * P)
            mch = mchs[g]
            pieces = (
                [(cc * C + ss * RG, RG, cc * NSUB + ss) for ss in range(NSUB)]
                if cc == NCH - 1
                else [(cc * C, C, cc * NSUB)]
            )
            for col0, width, w0 in pieces:
                nw = width // RG
                if width == C:
                    lg_t = lp.tile([P, C], f32, tag="lg")
                    gu_t = up.tile([P, C], f32, tag="gu")
                    x_t = xp.tile([P, C], f32, tag="x")
                else:
                    lg_t = sp2.tile([P, RG], f32, tag="lgs")
                    gu_t = sp2.tile([P, RG], f32, tag="gus")
                    x_t = sp2.tile([P, RG], f32, tag="xs")
                nc.sync.dma_start(lg_t[:], lg_d.ap()[rows, col0 : col0 + width])
                nc.sync.dma_start(gu_t[:], gu_d.ap()[rows, col0 : col0 + width])
                # in-place on ACT: u -> ln(u) -> ln(-ln(u))
                nc.scalar.activation(gu_t[:], gu_t[:], Act.Ln)
                nc.scalar.activation(gu_t[:], gu_t[:], Act.Ln, scale=-1.0)
                sub_eng = (
                    nc.vector if (width != C or cc in DVE_SUB) else nc.gpsimd
                )
                sub_eng.tensor_tensor(x_t[:], lg_t[:], gu_t[:], Alu.subtract)
                if RED == "dve3" and nw > 1:
                    nc.vector.tensor_reduce(
                        mch[:, w0 : w0 + nw],
                        x_t[:].rearrange("p (n c) -> p n c", c=RG),
                        mybir.AxisListType.X,
                        Alu.max,
                    )
                else:
                    for ss in range(nw):
                        nc.vector.tensor_reduce(
                            mch[:, w0 + ss : w0 + ss + 1],
                            x_t[:, ss * RG : (ss + 1) * RG],
                            mybir.AxisListType.X,
                            Alu.max,
                        )

        def emit_psg_gathers():
            """Token-side psg gathers for both groups.  Emitted after group
            0's chunk loop so the Pool queue head never blocks the early
            pool_max work on the sc2 round trip."""
            nc.gpsimd.dma_start(sc2_d.ap(), s2t[:])
            for g in range(GROUPS):
                bvec, lvec = bvecs[g], lvecs[g]
                sn = tp.tile([P, 2], i32, tag="sn")
                nc.gpsimd.indirect_dma_start(
                    out=sn[:],
                    out_offset=None,
                    in_=sc2_d.ap(),
                    in_offset=bass.IndirectOffsetOnAxis(ap=bvec[:, 0:1], axis=0),
                )
                # p = (l - shift + 512) & 511
                pv = tp.tile([P, 1], i32, tag="pv")
                nc.vector.tensor_tensor(pv[:], lvec[:], sn[:, 0:1], Alu.subtract)
                nc.vector.tensor_scalar(pv[:], pv[:], 512, None, Alu.add)
                nc.vector.tensor_tensor(pv[:], pv[:], c511p[:], Alu.bitwise_and)
                bsh = tp.tile([P, 1], i32, tag="bsh")
                nc.vector.tensor_scalar(bsh[:], bvec[:], 512, None, Alu.mult)
                # gather att[b, 511-p] : off = b*512 + 511 - p
                offa2 = tp.tile([P, 1], i32, tag="offa2")
                nc.vector.tensor_scalar(offa2[:], pv[:], -1, 511, Alu.mult, Alu.add)
                nc.vector.tensor_tensor(offa2[:], offa2[:], bsh[:], Alu.add)
                gA = tp.tile([P, 1], i32, tag="gA")
                nc.gpsimd.indirect_dma_start(
                    out=gA[:],
                    out_offset=None,
                    in_=att_flat,
                    in_offset=bass.IndirectOffsetOnAxis(ap=offa2[:, 0:1], axis=0),
                )
                # gather psg_input[b, p-1] (clamped; p==0 handled by select)
                offp = tp.tile([P, 1], i32, tag="offp")
                nc.vector.tensor_tensor(offp[:], bsh[:], pv[:], Alu.add)
                nc.vector.tensor_scalar(offp[:], offp[:], -1, 0, Alu.add, Alu.max)
                gP = tp.tile([P, 1], i32, tag="gP")
                nc.gpsimd.indirect_dma_start(
                    out=gP[:],
                    out_offset=None,
                    in_=psg_flat,
                    in_offset=bass.IndirectOffsetOnAxis(ap=offp[:, 0:1], axis=0),
                )
                eq0 = tp.tile([P, 1], i32, tag="eq0")
                nc.vector.tensor_scalar(eq0[:], pv[:], 0, None, Alu.is_equal)
                gPe = tp.tile([P, 1], i32, tag="gPe")
                nc.vector.select(gPe[:], eq0[:], ones_i[:], gP[:])
                tA = tp.tile([P, 1], i32, tag="tA")
                nc.vector.tensor_scalar(tA[:], gA[:], -1, 1, Alu.mult, Alu.add)
                id2 = tp.tile([P, 1], i32, tag="id2")
                nc.vector.tensor_tensor(id2[:], tA[:], gPe[:], Alu.mult)
                s2f = sm.tile([P, 1], f32, tag=f"s2f{g}")
                nc.vector.tensor_tensor(s2f[:], lvec[:], sn[:, 1:2], Alu.is_ge)
                e2 = sm.tile([P, D], f32, tag=f"e2_{g}")
                nc.gpsimd.indirect_dma_start(
                    out=e2[:],
                    out_offset=None,
                    in_=w_d.ap(),
                    in_offset=bass.IndirectOffsetOnAxis(ap=id2[:, 0:1], axis=0),
                )
                attf = sm.tile([P, 1], f32, tag=f"attf{g}")
                nc.vector.tensor_copy(attf[:], ams[g][:])
                e2s.append(e2)
                s2fs.append(s2f)
                s1parts.append(attf)

        def emit_phase_b(g):
            mch = mchs[g]
            # ---- winning window per row ----
            M_t = sm.tile([P, 1], f32, tag=f"M{g}")
            nc.vector.tensor_reduce(M_t[:], mch[:], mybir.AxisListType.X, Alu.max)
            M8 = sm.tile([P, 8], f32, tag=f"M8{g}")
            nc.vector.tensor_copy(M8[:], M_t[:, 0:1].to_broadcast([P, 8]))
            c8 = sm.tile([P, 8], u32, tag=f"c8{g}")
            nc.vector.max_index(c8[:], M8[:], mch[:])
            cst = sm.tile([P, 1], i32, tag=f"cst{g}")
            nc.vector.tensor_copy(cst[:], c8[:, 0:1])

            # ---- refetch winning window, exact argmax ----
            offA = tp.tile([P, 1], i32, tag="offA")
            nc.vector.scalar_tensor_tensor(
                offA[:], lrs[g][:], NCHR, cst[:], Alu.mult, Alu.add
            )
            lgr = rf.tile([P, RG], f32, tag="lgr")
            nc.gpsimd.indirect_dma_start(
                out=lgr[:],
                out_offset=None,
                in_=lg_view,
                in_offset=bass.IndirectOffsetOnAxis(ap=offA[:, 0:1], axis=0),
            )
            gur = rf.tile([P, RG], f32, tag="gur")
            nc.gpsimd.indirect_dma_start(
                out=gur[:],
                out_offset=None,
                in_=gu_view,
                in_offset=bass.IndirectOffsetOnAxis(ap=offA[:, 0:1], axis=0),
            )
            nc.scalar.activation(gur[:], gur[:], Act.Ln)
            nc.scalar.activation(gur[:], gur[:], Act.Ln, scale=-1.0)
            nc.vector.tensor_tensor(lgr[:], lgr[:], gur[:], Alu.subtract)
            li8 = sm.tile([P, 8], u32, tag=f"li8{g}")
            nc.vector.max_index(li8[:], M8[:], lgr[:])
            lii = sm.tile([P, 1], i32, tag=f"lii{g}")
            nc.vector.tensor_copy(lii[:], li8[:, 0:1])
            gidx = sm.tile([P, 1], i32, tag=f"gidx{g}")
            nc.vector.scalar_tensor_tensor(gidx[:], cst[:], RG, lii[:], Alu.mult, Alu.add)

            # ---- gather 1: argmax embedding ----
            v1f = tp.tile([P, 1], f32, tag="v1f")
            nc.vector.tensor_scalar(v1f[:], gidx[:], AV, None, Alu.is_lt)
            s1 = tp.tile([P, 1], f32, tag="s1")
            nc.vector.tensor_tensor(s1[:], v1f[:], s1parts[g][:], Alu.mult)
            idx1c = tp.tile([P, 1], i32, tag="idx1c")
            nc.vector.tensor_scalar(idx1c[:], gidx[:], AV - 1, None, Alu.min)
            e1 = ep.tile([P, D], f32, tag="e1")
            nc.gpsimd.indirect_dma_start(
                out=e1[:],
                out_offset=None,
                in_=w_d.ap(),
                in_offset=bass.IndirectOffsetOnAxis(ap=idx1c[:, 0:1], axis=0),
            )

            # ---- combine (store happens at the very end, on SP) ----
            o1 = ep.tile([P, D], f32, tag="o1")
            nc.vector.tensor_scalar(o1[:], e1[:], s1[:, 0:1], None, Alu.mult)
            o2 = sm.tile([P, D], f32, tag=f"o2_{g}")
            nc.vector.scalar_tensor_tensor(
                o2[:], e2s[g][:], s2fs[g][:, 0:1], o1[:], Alu.mult, Alu.add
            )
            o2s.append(o2)

        # ---------------- main schedule ----------------
        for g in range(GROUPS):
            for cc in range(NCH):
                emit_chunk(g, cc)
            if g == 0:
                emit_psg_gathers()
            emit_phase_b(g)

        for g in range(GROUPS):
            rows = slice(g * P, (g + 1) * P)
            nc.sync.dma_start(out_d.ap()[rows, :], o2s[g][:])

    nc.compile()
    return nc


def _get_program():
    if "nc" not in _CACHE:
        _CACHE["nc"] = _build_program()
    return _CACHE["nc"]


def make_in_maps(logits, gumbel_u, word_embeddings, rwrt_attention, psg_input):
    lg = np.ascontiguousarray(np.asarray(logits, np.float32).reshape(R, V))
    gu = np.ascontiguousarray(np.asarray(gumbel_u, np.float32).reshape(R, V))
    W = np.ascontiguousarray(np.asarray(word_embeddings, np.float32))
    att = np.ascontiguousarray(np.asarray(rwrt_attention, np.int32))
    psg = np.ascontiguousarray(np.asarray(psg_input, np.int32))
    att_flat = att.reshape(R)
    in_maps = []
    for c in range(NCORES):
        r0 = c * RC
        rows = np.arange(r0, r0 + RC, dtype=np.int32)
        in_maps.append(
            {
                "logits": lg[r0 : r0 + RC],
                "gumbel": gu[r0 : r0 + RC],
                "wemb": W,
                "att": att,
                "psg": psg,
                "bcol": np.ascontiguousarray((rows >> 9).reshape(RC, 1)),
                "lcol": np.ascontiguousarray((rows & 511).reshape(RC, 1)),
                "lrow": np.arange(RC, dtype=np.int32).reshape(RC, 1),
                "attmy": np.ascontiguousarray(
                    att_flat[r0 : r0 + RC].reshape(RC, 1)
                ),
            }
        )
    return in_maps


def kernel(logits, gumbel_u, word_embeddings, rwrt_attention, psg_input):
    from concourse import bass_utils

    nc = _get_program()
    in_maps = make_in_maps(logits, gumbel_u, word_embeddings, rwrt_attention, psg_input)
    tmpdir = os.environ.get("BASS_KERNEL_TMPDIR") or None
    res = bass_utils.run_bass_kernel_spmd(
        nc, in_maps, core_ids=list(range(NCORES)), tmpdir=tmpdir
    )
    LAST["exec_time_ns"] = res.exec_time_ns
    LAST["tmpdir"] = tmpdir
    if res.instructions_and_trace is not None:
        LAST["trace_path"] = res.instructions_and_trace[1]
    out = np.concatenate([res.results[c]["out"] for c in range(NCORES)], axis=0)
    return out.reshape(B, L, D).astype(np.float32)


# revision 19
# speedup vs baseline: 1.1502x; 1.0138x over previous
"""Trainium2 Bass kernel for nn_End2End_10316511445013 (embedding_lookup).

Math being implemented (see the reference nn.Module):
  1. x = logits + g,  g = -ln(-ln(u))          [B,L,V]
  2. In fp32 the straight-through one-hot  y = y_hard + y_soft - y_soft  is
     *exactly* alpha * one_hot(argmax(x)) with alpha = fl(fl(1+s)-s) = 1 +/- 2^-23,
     so the einsum with the embedding table is exactly an embedding row gather
     scaled by alpha (~1, error < 1.2e-7 relative -> we use 1).
  3. inputs_embeds[b,l] = att[b,l] * (idx < AV) * W[idx],  idx = argmax_v x[b,l,:]
  4. psg path: trunc_ids / flag index logic on [B,L] int tensors, then a second
     row gather of W, all computed on-device with small DVE ops + indirect DMA.

Distribution: data-parallel over the B*L = 2048 rows; 256 rows per core; the
94MB embedding table is replicated to every core.  Per core we stream the
(logits, gumbel) shard in [128, 2008] chunks.  Engine assignment keeps DMA the
only saturated resource:
  SP (sync)  : only the 64 streaming chunk DMAs + the 2 output stores (emitted
               last), so its in-order HWDGE queue never head-blocks the stream.
  ACT        : u -> ln(u) -> ln(-ln(u)) in place (2 passes per chunk).
  DVE        : x = logits - ln(-ln(u)) subtract, psg index math, max_index.
  Pool       : pool_max over [128, 4, 502] windows -> per-window row max, plus
               all SWDGE small loads / indirect gathers / output stores.
The last chunk of each group is split into 4 window-sized pieces so the
pipeline drain latency at the group boundary / kernel tail is ~3us not ~10us.
The argmax index is recovered by refetching only the winning 502-wide window
per row (indirect DMA) and running max_index on it.
"""

import os
import sys

import numpy as np

sys.path.insert(0, "/opt/trn_rl_repo")

B, L, V, AV, D = 4, 512, 32128, 32000, 768
R = B * L            # 2048 tokens total
NCORES = 8
RC = R // NCORES     # 256 tokens per core
P = 128              # partitions
GROUPS = RC // P     # 2 groups of 128 tokens
NCH = 16             # vocab chunks per row (DMA granularity)
C = V // NCH         # 2008
NSUB = 4             # max-reduce windows per chunk
RG = C // NSUB       # 502: reduce granularity = phase-B refetch window
NCHR = NCH * NSUB    # 64 reduce windows per row
NEG_BIG = -3.0e38
# windowed reduce: "dve3" = one 3D tensor_reduce per chunk, "dve" = per-window
RED = os.environ.get("KERNEL_RED", "dve3")
# full chunks whose subtract runs on DVE instead of Pool (Pool absorbs the
# rest; these slots sit where Pool is busy with SWDGE bursts, and the last
# full chunk stays on DVE so no 4.7us Pool-TT latency sits near the tail)
DVE_SUB = {0, 3, 6, 9, 12, 13}
# chunks streamed/processed as 2 half-chunks of [128, 1004] so the pipeline
# drain at the group boundary / kernel tail is short
TAIL_CHUNKS = {14, 15}
HC = C * NSUB // (2 * NSUB)  # 1004: half-chunk width

_CACHE = {}
LAST = {}            # exec_time_ns etc. for test harness introspection


def _build_program():
    from contextlib import ExitStack

    import concourse.bass as bass
    import concourse.tile as tile
    from concourse import bacc, mybir

    f32 = mybir.dt.float32
    i32 = mybir.dt.int32
    u32 = mybir.dt.uint32
    Alu = mybir.AluOpType
    Act = mybir.ActivationFunctionType

    nc = bacc.Bacc(
        "TRN2",
        target_bir_lowering=False,
        debug=False,
        enable_asserts=True,
        num_devices=NCORES,
    )

    lg_d = nc.dram_tensor("logits", [RC, V], f32, kind="ExternalInput")
    gu_d = nc.dram_tensor("gumbel", [RC, V], f32, kind="ExternalInput")
    w_d = nc.dram_tensor("wemb", [AV, D], f32, kind="ExternalInput")
    att_d = nc.dram_tensor("att", [B, L], i32, kind="ExternalInput")
    psg_d = nc.dram_tensor("psg", [B, L], i32, kind="ExternalInput")
    bc_d = nc.dram_tensor("bcol", [RC, 1], i32, kind="ExternalInput")
    lc_d = nc.dram_tensor("lcol", [RC, 1], i32, kind="ExternalInput")
    lr_d = nc.dram_tensor("lrow", [RC, 1], i32, kind="ExternalInput")
    am_d = nc.dram_tensor("attmy", [RC, 1], i32, kind="ExternalInput")
    out_d = nc.dram_tensor("out", [RC, D], f32, kind="ExternalOutput")
    sc2_d = nc.dram_tensor("scratch2", [B, 2], i32, kind="Internal")

    # flat views for indirect row gathers (offset must be 0)
    lg_view = lg_d.ap().rearrange("r (n c) -> (r n) c", c=RG)
    gu_view = gu_d.ap().rearrange("r (n c) -> (r n) c", c=RG)
    att_flat = att_d.ap().rearrange("b (l o) -> (b l) o", o=1)
    psg_flat = psg_d.ap().rearrange("b (l o) -> (b l) o", o=1)

    with tile.TileContext(nc) as tc, ExitStack() as ctx:
        sm = ctx.enter_context(tc.tile_pool(name="small", bufs=1))
        lp = ctx.enter_context(tc.tile_pool(name="lg", bufs=6))
        up = ctx.enter_context(tc.tile_pool(name="gu", bufs=6))
        xp = ctx.enter_context(tc.tile_pool(name="x", bufs=4))
        sp2 = ctx.enter_context(tc.tile_pool(name="spl", bufs=2))
        rf = ctx.enter_context(tc.tile_pool(name="rf", bufs=2))
        ep = ctx.enter_context(tc.tile_pool(name="emb", bufs=2))
        tp = ctx.enter_context(tc.tile_pool(name="tok", bufs=2))

        # ---------------- small input loads: Pool SWDGE, no waits ----------
        A_t = sm.tile([B, L], i32, tag="psgA")
        nc.gpsimd.dma_start(A_t[:], att_d.ap())
        P_t = sm.tile([B, L], i32, tag="psgP")
        nc.gpsimd.dma_start(P_t[:], psg_d.ap())
        bvecs, lvecs, ams, lrs = [], [], [], []
        for g in range(GROUPS):
            rows = slice(g * P, (g + 1) * P)
            bvec = sm.tile([P, 1], i32, tag=f"bvec{g}")
            nc.gpsimd.dma_start(bvec[:], bc_d.ap()[rows, :])
            lvec = sm.tile([P, 1], i32, tag=f"lvec{g}")
            nc.gpsimd.dma_start(lvec[:], lc_d.ap()[rows, :])
            am_t = sm.tile([P, 1], i32, tag=f"am{g}")
            nc.gpsimd.dma_start(am_t[:], am_d.ap()[rows, :])
            lr_t = sm.tile([P, 1], i32, tag=f"lr{g}")
            nc.gpsimd.dma_start(lr_t[:], lr_d.ap()[rows, :])
            bvecs.append(bvec)
            lvecs.append(lvec)
            ams.append(am_t)
            lrs.append(lr_t)

        # ---------------- psg index stage on [B, 512] (DVE) ----------------
        LI_t = sm.tile([B, L], i32, tag="psgLI")
        nc.gpsimd.iota(LI_t[:], [[1, L]], base=0, channel_multiplier=0)

        shift = sm.tile([B, 1], i32, tag="shift")
        with nc.allow_low_precision(reason="exact int32 sum of 0/1 mask"):
            nc.vector.tensor_reduce(shift[:], A_t[:], mybir.AxisListType.X, Alu.add)

        FA = sm.tile([B, L], i32, tag="FA")  # FA[j] = att[511-j]
        nc.vector.tensor_copy(FA[:], A_t[:, ::-1])
        PR = sm.tile([B, L], i32, tag="PR")  # roll(psg,1) with [:,0]=1
        nc.vector.memset(PR[:, 0:1], 1)
        nc.vector.tensor_copy(PR[:, 1:L], P_t[:, 0 : L - 1])

        t1 = sm.tile([B, L], i32, tag="t1")
        nc.vector.tensor_scalar(t1[:], FA[:], 0, None, Alu.is_equal)
        t2 = sm.tile([B, L], i32, tag="t2")
        nc.vector.tensor_scalar(t2[:], PR[:], 0, None, Alu.not_equal)
        nzm = sm.tile([B, L], i32, tag="nzm")
        nc.vector.tensor_tensor(nzm[:], t1[:], t2[:], Alu.mult)

        # v(j) = (j + shift) & 511 : position in trunc space
        c511b = sm.tile([B, 1], i32, tag="c511b")
        nc.vector.memset(c511b[:], 511)
        v_t = sm.tile([B, L], i32, tag="v")
        nc.vector.tensor_tensor(
            v_t[:], LI_t[:], shift[:, 0:1].to_broadcast([B, L]), Alu.add
        )
        nc.vector.tensor_tensor(
            v_t[:], v_t[:], c511b[:, 0:1].to_broadcast([B, L]), Alu.bitwise_and
        )
        # cand = nz ? v : 9999  ==  (v - 9999)*nz + 9999
        c1 = sm.tile([B, L], i32, tag="c1")
        nc.vector.scalar_tensor_tensor(c1[:], v_t[:], 9999, nzm[:], Alu.subtract, Alu.mult)
        cand = sm.tile([B, L], i32, tag="cand")
        nc.vector.tensor_scalar(cand[:], c1[:], 9999, None, Alu.add)
        nzpos = sm.tile([B, 1], i32, tag="nzpos")
        nc.vector.tensor_reduce(nzpos[:], cand[:], mybir.AxisListType.X, Alu.min)

        s2t = sm.tile([B, 2], i32, tag="s2t")
        nc.vector.tensor_copy(s2t[:, 0:1], shift[:])
        nc.vector.tensor_copy(s2t[:, 1:2], nzpos[:])

        ones_i = sm.tile([P, 1], i32, tag="ones")
        nc.vector.memset(ones_i[:], 1)
        c511p = sm.tile([P, 1], i32, tag="c511p")
        nc.vector.memset(c511p[:], 511)

        mchs = []
        for g in range(GROUPS):
            mch_g = sm.tile([P, NCHR], f32, tag=f"mch{g}")
            mchs.append(mch_g)

        e2s, s2fs, s1parts, o2s = [], [], [], []

        def emit_chunk(g, cc):
            """Stream + process one [128, 2008] chunk; the last chunk of a
            group is split into 4 window-sized pieces for drain latency.
            The subtract runs on Pool (software TT) for most chunks and on
            DVE for DVE_SUB chunks + the split pieces, so neither engine's
            per-slot work exceeds the 5.7us DMA slot."""
            rows = slice(g * P, (g + 1) * P)
            mch = mchs[g]
            pieces = (
                [(cc * C + h * HC, HC, cc * NSUB + h * 2) for h in range(2)]
                if cc in TAIL_CHUNKS
                else [(cc * C, C, cc * NSUB)]
            )
            for col0, width, w0 in pieces:
                nw = width // RG
                if width == C:
                    lg_t = lp.tile([P, C], f32, tag="lg")
                    gu_t = up.tile([P, C], f32, tag="gu")
                    x_t = xp.tile([P, C], f32, tag="x")
                else:
                    lg_t = sp2.tile([P, HC], f32, tag="lgs")
                    gu_t = sp2.tile([P, HC], f32, tag="gus")
                    x_t = sp2.tile([P, HC], f32, tag="xs")
                nc.sync.dma_start(lg_t[:], lg_d.ap()[rows, col0 : col0 + width])
                nc.sync.dma_start(gu_t[:], gu_d.ap()[rows, col0 : col0 + width])
                # in-place on ACT: u -> ln(u) -> ln(-ln(u))
                nc.scalar.activation(gu_t[:], gu_t[:], Act.Ln)
                nc.scalar.activation(gu_t[:], gu_t[:], Act.Ln, scale=-1.0)
                sub_eng = (
                    nc.vector if (width != C or cc in DVE_SUB) else nc.gpsimd
                )
                sub_eng.tensor_tensor(x_t[:], lg_t[:], gu_t[:], Alu.subtract)
                if RED == "dve3" and nw > 1:
                    nc.vector.tensor_reduce(
                        mch[:, w0 : w0 + nw],
                        x_t[:].rearrange("p (n c) -> p n c", c=RG),
                        mybir.AxisListType.X,
                        Alu.max,
                    )
                else:
                    for ss in range(nw):
                        nc.vector.tensor_reduce(
                            mch[:, w0 + ss : w0 + ss + 1],
                            x_t[:, ss * RG : (ss + 1) * RG],
                            mybir.AxisListType.X,
                            Alu.max,
                        )

        def emit_psg_gathers():
            """Token-side psg gathers for both groups.  Emitted after group
            0's chunk loop so the Pool queue head never blocks the early
            pool_max work on the sc2 round trip."""
            nc.gpsimd.dma_start(sc2_d.ap(), s2t[:])
            for g in range(GROUPS):
                bvec, lvec = bvecs[g], lvecs[g]
                sn = tp.tile([P, 2], i32, tag="sn")
                nc.gpsimd.indirect_dma_start(
                    out=sn[:],
                    out_offset=None,
                    in_=sc2_d.ap(),
                    in_offset=bass.IndirectOffsetOnAxis(ap=bvec[:, 0:1], axis=0),
                )
                # p = (l - shift + 512) & 511
                pv = tp.tile([P, 1], i32, tag="pv")
                nc.vector.tensor_tensor(pv[:], lvec[:], sn[:, 0:1], Alu.subtract)
                nc.vector.tensor_scalar(pv[:], pv[:], 512, None, Alu.add)
                nc.vector.tensor_tensor(pv[:], pv[:], c511p[:], Alu.bitwise_and)
                bsh = tp.tile([P, 1], i32, tag="bsh")
                nc.vector.tensor_scalar(bsh[:], bvec[:], 512, None, Alu.mult)
                # gather att[b, 511-p] : off = b*512 + 511 - p
                offa2 = tp.tile([P, 1], i32, tag="offa2")
                nc.vector.tensor_scalar(offa2[:], pv[:], -1, 511, Alu.mult, Alu.add)
                nc.vector.tensor_tensor(offa2[:], offa2[:], bsh[:], Alu.add)
                gA = tp.tile([P, 1], i32, tag="gA")
                nc.gpsimd.indirect_dma_start(
                    out=gA[:],
                    out_offset=None,
                    in_=att_flat,
                    in_offset=bass.IndirectOffsetOnAxis(ap=offa2[:, 0:1], axis=0),
                )
                # gather psg_input[b, p-1] (clamped; p==0 handled by select)
                offp = tp.tile([P, 1], i32, tag="offp")
                nc.vector.tensor_tensor(offp[:], bsh[:], pv[:], Alu.add)
                nc.vector.tensor_scalar(offp[:], offp[:], -1, 0, Alu.add, Alu.max)
                gP = tp.tile([P, 1], i32, tag="gP")
                nc.gpsimd.indirect_dma_start(
                    out=gP[:],
                    out_offset=None,
                    in_=psg_flat,
                    in_offset=bass.IndirectOffsetOnAxis(ap=offp[:, 0:1], axis=0),
                )
                eq0 = tp.tile([P, 1], i32, tag="eq0")
                nc.vector.tensor_scalar(eq0[:], pv[:], 0, None, Alu.is_equal)
                gPe = tp.tile([P, 1], i32, tag="gPe")
                nc.vector.select(gPe[:], eq0[:], ones_i[:], gP[:])
                tA = tp.tile([P, 1], i32, tag="tA")
                nc.vector.tensor_scalar(tA[:], gA[:], -1, 1, Alu.mult, Alu.add)
                id2 = tp.tile([P, 1], i32, tag="id2")
                nc.vector.tensor_tensor(id2[:], tA[:], gPe[:], Alu.mult)
                s2f = sm.tile([P, 1], f32, tag=f"s2f{g}")
                nc.vector.tensor_tensor(s2f[:], lvec[:], sn[:, 1:2], Alu.is_ge)
                e2 = sm.tile([P, D], f32, tag=f"e2_{g}")
                nc.gpsimd.indirect_dma_start(
                    out=e2[:],
                    out_offset=None,
                    in_=w_d.ap(),
                    in_offset=bass.IndirectOffsetOnAxis(ap=id2[:, 0:1], axis=0),
                )
                attf = sm.tile([P, 1], f32, tag=f"attf{g}")
                nc.vector.tensor_copy(attf[:], ams[g][:])
                # pre-masked psg embedding: the tail then needs only one
                # multiply-add to combine both paths
                o2e = sm.tile([P, D], f32, tag=f"o2e{g}")
                nc.vector.tensor_scalar(
                    o2e[:], e2[:], s2f[:, 0:1], None, Alu.mult
                )
                e2s.append(o2e)
                s2fs.append(s2f)
                s1parts.append(attf)

        def emit_phase_b(g):
            mch = mchs[g]
            # ---- winning window per row ----
            M_t = sm.tile([P, 1], f32, tag=f"M{g}")
            nc.vector.tensor_reduce(M_t[:], mch[:], mybir.AxisListType.X, Alu.max)
            M8 = sm.tile([P, 8], f32, tag=f"M8{g}")
            nc.vector.tensor_copy(M8[:], M_t[:, 0:1].to_broadcast([P, 8]))
            c8 = sm.tile([P, 8], u32, tag=f"c8{g}")
            nc.vector.max_index(c8[:], M8[:], mch[:])
            cst = sm.tile([P, 1], i32, tag=f"cst{g}")
            nc.vector.tensor_copy(cst[:], c8[:, 0:1])

            # ---- refetch winning window, exact argmax ----
            offA = tp.tile([P, 1], i32, tag="offA")
            nc.vector.scalar_tensor_tensor(
                offA[:], lrs[g][:], NCHR, cst[:], Alu.mult, Alu.add
            )
            # gur gather first: the ACT Ln passes only need gur, so its
            # descriptors go out ahead of lgr's
            gur = rf.tile([P, RG], f32, tag="gur")
            nc.gpsimd.indirect_dma_start(
                out=gur[:],
                out_offset=None,
                in_=gu_view,
                in_offset=bass.IndirectOffsetOnAxis(ap=offA[:, 0:1], axis=0),
            )
            lgr = rf.tile([P, RG], f32, tag="lgr")
            nc.gpsimd.indirect_dma_start(
                out=lgr[:],
                out_offset=None,
                in_=lg_view,
                in_offset=bass.IndirectOffsetOnAxis(ap=offA[:, 0:1], axis=0),
            )
            nc.scalar.activation(gur[:], gur[:], Act.Ln)
            nc.scalar.activation(gur[:], gur[:], Act.Ln, scale=-1.0)
            nc.vector.tensor_tensor(lgr[:], lgr[:], gur[:], Alu.subtract)
            li8 = sm.tile([P, 8], u32, tag=f"li8{g}")
            nc.vector.max_index(li8[:], M8[:], lgr[:])
            lii = sm.tile([P, 1], i32, tag=f"lii{g}")
            nc.vector.tensor_copy(lii[:], li8[:, 0:1])
            gidx = sm.tile([P, 1], i32, tag=f"gidx{g}")
            nc.vector.scalar_tensor_tensor(gidx[:], cst[:], RG, lii[:], Alu.mult, Alu.add)

            # ---- gather 1: argmax embedding ----
            v1f = tp.tile([P, 1], f32, tag="v1f")
            nc.vector.tensor_scalar(v1f[:], gidx[:], AV, None, Alu.is_lt)
            s1 = tp.tile([P, 1], f32, tag="s1")
            nc.vector.tensor_tensor(s1[:], v1f[:], s1parts[g][:], Alu.mult)
            idx1c = tp.tile([P, 1], i32, tag="idx1c")
            nc.vector.tensor_scalar(idx1c[:], gidx[:], AV - 1, None, Alu.min)
            e1 = ep.tile([P, D], f32, tag="e1")
            nc.gpsimd.indirect_dma_start(
                out=e1[:],
                out_offset=None,
                in_=w_d.ap(),
                in_offset=bass.IndirectOffsetOnAxis(ap=idx1c[:, 0:1], axis=0),
            )

            # ---- combine (store happens at the very end, on SP) ----
            o2 = sm.tile([P, D], f32, tag=f"o2_{g}")
            nc.vector.scalar_tensor_tensor(
                o2[:], e1[:], s1[:, 0:1], e2s[g][:], Alu.mult, Alu.add
            )
            o2s.append(o2)

        # ---------------- main schedule ----------------
        for g in range(GROUPS):
            for cc in range(NCH):
                emit_chunk(g, cc)
            if g == 0:
                emit_psg_gathers()
            emit_phase_b(g)

        for g in range(GROUPS):
            rows = slice(g * P, (g + 1) * P)
            nc.sync.dma_start(out_d.ap()[rows, :], o2s[g][:])

    nc.compile()
    return nc


def _get_program():
    if "nc" not in _CACHE:
        _CACHE["nc"] = _build_program()
    return _CACHE["nc"]


def make_in_maps(logits, gumbel_u, word_embeddings, rwrt_attention, psg_input):
    lg = np.ascontiguousarray(np.asarray(logits, np.float32).reshape(R, V))
    gu = np.ascontiguousarray(np.asarray(gumbel_u, np.float32).reshape(R, V))
    W = np.ascontiguousarray(np.asarray(word_embeddings, np.float32))
    att = np.ascontiguousarray(np.asarray(rwrt_attention, np.int32))
    psg = np.ascontiguousarray(np.asarray(psg_input, np.int32))
    att_flat = att.reshape(R)
    in_maps = []
    for c in range(NCORES):
        r0 = c * RC
        rows = np.arange(r0, r0 + RC, dtype=np.int32)
        in_maps.append(
            {
                "logits": lg[r0 : r0 + RC],
                "gumbel": gu[r0 : r0 + RC],
                "wemb": W,
                "att": att,
                "psg": psg,
                "bcol": np.ascontiguousarray((rows >> 9).reshape(RC, 1)),
                "lcol": np.ascontiguousarray((rows & 511).reshape(RC, 1)),
                "lrow": np.arange(RC, dtype=np.int32).reshape(RC, 1),
                "attmy": np.ascontiguousarray(
                    att_flat[r0 : r0 + RC].reshape(RC, 1)
                ),
            }
        )
    return in_maps


def kernel(logits, gumbel_u, word_embeddings, rwrt_attention, psg_input):
    from concourse import bass_utils

    nc = _get_program()
    in_maps = make_in_maps(logits, gumbel_u, word_embeddings, rwrt_attention, psg_input)
    tmpdir = os.environ.get("BASS_KERNEL_TMPDIR") or None
    res = bass_utils.run_bass_kernel_spmd(
        nc, in_maps, core_ids=list(range(NCORES)), tmpdir=tmpdir
    )
    LAST["exec_time_ns"] = res.exec_time_ns
    LAST["tmpdir"] = tmpdir
    if res.instructions_and_trace is not None:
        LAST["trace_path"] = res.instructions_and_trace[1]
    out = np.concatenate([res.results[c]["out"] for c in range(NCORES)], axis=0)
    return out.reshape(B, L, D).astype(np.float32)
